# revision 48
# baseline (speedup 1.0000x reference)
"""Trainium2 Bass kernel for nn_NNModel2 (2x NNConv GNN + pooled MLP readout).

Self-contained: accepts FULL inputs, returns the FULL [256, 1] output.

Sharding (one collective total):
  A balanced node partition pi (512 nodes/core, greedy over in/out degree)
  defines ownership. conv1 edges are sharded by pi(dst): each core scatter-adds
  msg1 into its own h1 slice locally. conv2 edges are sharded by pi(src): each
  core gathers h1[src] from its local h1 slice. conv2 has no nonlinearity
  before global mean pooling, so the msg2 scatter-by-dst + pool-by-graph is
  fused into a single pool-by-graph(dst) one-hot matmul; per-core pooled
  partial sums [256, 256] are combined with ONE ReduceScatter (each core gets
  32 graphs), each core runs the readout MLP on its 32 graphs, and the host
  concatenates the 8x[32,1] outputs.

Math (per NNConv layer, aggr='add'):
    w_e  = (edge_attr @ nn_w + nn_b).reshape(E, I, O)
    msg  = einsum('ei,eio->eo', x[src], w_e)
    out  = segment_sum(msg, dst, N) + x @ root_w + bias
restructured as one dense matmul over z:
    z[e, (k,i)] = edge_attr[e,k] * x[src[e], i]
    msg = z @ W' + x[src] @ B';  W'[(k,i), o] = nn_w[k, i*O+o]

All small constants/index tables are host-packed into two tensors (one f32,
one i16) loaded with single DMAs; small weight matrices ride in the f32 pack
and are converted to bf16 on-device in one op.
"""

import sys

sys.path.insert(0, "/opt/trn_rl_repo")

import numpy as np

from concourse import bacc, bass, mybir
import concourse.tile as tile
from concourse import bass_utils

P = 128
NCORES = 8
N_NODES = 4096
N_EDGES = 8192
N_GRAPHS = 256
DN = 64
DE = 32
H = 256
NSH = N_NODES // NCORES  # 512
NT = NSH // P  # 4
GPC = N_GRAPHS // NCORES  # 32 graphs per core after ReduceScatter

F32 = mybir.dt.float32
F32R = mybir.dt.float32r
BF16 = mybir.dt.bfloat16
I16 = mybir.dt.int16
F8 = mybir.dt.float8e4
DR = mybir.MatmulPerfMode.DoubleRow
AF = mybir.ActivationFunctionType
ALU = mybir.AluOpType

_cache = {}


def _wrap_idx(idx, n):
    idx = np.asarray(idx, dtype=np.int16)
    assert idx.shape == (n,) and n % 16 == 0
    return np.tile(idx.reshape(n // 16, 16).T, (8, 1)).copy()


def _lay(e1, e2):
    ET1, ET2 = e1 // P, e2 // P
    L = {}
    c = 0
    for name, w in [
        ("io512", NSH), ("iog", N_GRAPHS), ("dstl", ET1), ("gl", ET2),
        ("batchl", NT), ("invc", 1), ("l1b", 1),
        ("b1", H), ("b2", H), ("cnt", N_GRAPHS), ("l2b", 1),
    ]:
        L[name] = c
        c += w
    L["wb0"] = c
    for name, w in [
        ("b1p", H), ("r1w", H), ("b2p", 2 * H), ("r2w", 2 * H),
        ("l1w", H), ("l2w", 1), ("ident", P), ("ivd", GPC),
    ]:
        L[name] = c
        c += w
    L["cw"] = c
    ci = 0
    for name, w in [
        ("src2", e2 // 16), ("loc", NSH // 16),
    ]:
        L[name] = ci
        ci += w
    L["iw"] = ci
    return L


def _build(e1, e2, upto="full"):
    ET1 = e1 // P
    ET2 = e2 // P
    L = _lay(e1, e2)
    nc = bacc.Bacc(num_devices=NCORES)

    x1T = nc.dram_tensor("x1T", [P, e1], F32, kind="ExternalInput")
    xoT = nc.dram_tensor("xoT", [DN, NSH], F32, kind="ExternalInput")
    a1T = nc.dram_tensor("a1T", [DE, e1], F32, kind="ExternalInput")
    a2T = nc.dram_tensor("a2T", [DE, e2], F32, kind="ExternalInput")
    nn1_w = nc.dram_tensor("nn1_w", [DE, DN * H], F32, kind="ExternalInput")
    nn2_w = nc.dram_tensor("nn2_w", [DE, H * H], F32, kind="ExternalInput")
    cpackA = nc.dram_tensor("cpackA", [P, L["wb0"]], F32, kind="ExternalInput")
    cpackB = nc.dram_tensor("cpackB", [P, L["cw"] - L["wb0"]], F32, kind="ExternalInput")
    ipack = nc.dram_tensor("ipack", [P, L["iw"]], I16, kind="ExternalInput")
    out = nc.dram_tensor("out", [GPC, 1], F32, kind="ExternalOutput")

    def dbg_out(name, shape):
        return nc.dram_tensor(name, shape, F32, kind="ExternalOutput")

    rg = [list(range(NCORES))]
    ST = {"h1": 2, "pool": 4, "full": 99}[upto]
    WB0 = L["wb0"]
    WBW = L["cw"] - WB0

    with tile.TileContext(nc, num_cores=NCORES) as tc:
        with (
            tc.tile_pool(name="const", bufs=1) as cp,
            tc.tile_pool(name="work", bufs=3) as wp,
            tc.tile_pool(name="big", bufs=1) as bp,
            tc.tile_pool(name="dram", bufs=1, space="DRAM") as dr,
        ):
            cpk = cp.tile([P, L["wb0"]], F32, name="cpk")
            cpkB = cp.tile([P, WBW], F32, name="cpkB")
            wbf = cp.tile([P, WBW], BF16, name="wbf")
            ipk = cp.tile([P, L["iw"]], I16, name="ipk")

            def CF(name, w0, w, p0=0, p1=P):
                return cpk[p0:p1, L[name] + w0 : L[name] + w0 + w]

            def WF(name, w0, w, p0=0, p1=P):
                c0 = L[name] - WB0 + w0
                return wbf[p0:p1, c0 : c0 + w]

            def IF(name, w):
                return ipk[:, L[name] : L[name] + w]

            with tc.tile_pool(name="staging", bufs=2) as stp:
                # ---- SP queue: background loads in consumption order
                nc.sync.dma_start(out=ipk[:], in_=ipack[:])

                # host-sharded edge attrs (already transposed + col-permuted):
                # load f32, convert to bf16, write back for the broadcast DMAs
                a1f = stp.tile([DE, e1], F32, tag="a1f", bufs=1)
                nc.sync.dma_start(out=a1f[:], in_=a1T[:])
                a1b = stp.tile([DE, e1], BF16, tag="a1b", bufs=1)
                nc.vector.tensor_copy(out=a1b[:], in_=a1f[:])
                attrT1_dram = dr.tile([DE, e1], BF16)
                nc.sync.dma_start(out=attrT1_dram[:], in_=a1b[:])

                # host-sharded x: x[src].T (pre-duplicated to 128 rows) and
                # own-node x.T — just load + convert
                xsrcT = cp.tile([P, 1, e1], BF16)
                x1f = stp.tile([P, e1], F32, tag="x1f", bufs=1)
                nc.sync.dma_start(out=x1f[:], in_=x1T[:])
                nc.vector.tensor_copy(out=xsrcT[:, 0, :], in_=x1f[:])

                # bcb holds bc1 (rows 0:16) during conv1, then is overwritten
                # by bc2 (rows 0:32) for conv2 — bc2 DMAs are issued after the
                # conv1 z-loop so the framework serializes them behind bc1's
                # last reads.
                em = max(e1, e2)
                bcb = [
                    bp.tile([P, 8, em], BF16, name=f"bcb{i}") for i in range(4)
                ]
                # bc1 in quarter chunks interleaved with w1 staged chunks:
                # [p<64]: attr_perm[e, t] = attr[e, 2t]; [p>=64]: attr[e, 2t+1]
                w1sb = cp.tile([P, 16, H], BF16)
                w1_src = nn1_w.rearrange("(t k2) (i o) -> (k2 i) t o", k2=2, o=H)
                sink = dr.tile([1, 4], BF16, name="sink")

                def sp_block(tile_i, slot):
                    # tiny SBUF->DRAM DMA that waits on bc1 content; since the
                    # SP queue is in-order, every later SP DMA queues behind it
                    nc.sync.dma_start(
                        out=sink[0:1, slot : slot + 1], in_=bcb[tile_i][0:1, 0, 0:1]
                    )

                def bc1_chunk(half):
                    t0, t1 = 8 * half, 8 * (half + 1)
                    nc.sync.dma_start(
                        out=bcb[half][0:64, :, 0:e1],
                        in_=attrT1_dram[t0:t1, :].partition_broadcast(64),
                    )
                    nc.sync.dma_start(
                        out=bcb[half][64:128, :, 0:e1],
                        in_=attrT1_dram[16 + t0 : 16 + t1, :].partition_broadcast(64),
                    )

                def w1_chunk(c):
                    st1 = stp.tile([P, 4, H], F32, tag="wst", name=f"w1st{c}")
                    nc.sync.dma_start(
                        out=st1[:], in_=w1_src[:, 4 * c : 4 * (c + 1), :]
                    )
                    nc.scalar.activation(
                        out=w1sb[:, 4 * c : 4 * (c + 1), :], in_=st1[:],
                        func=AF.Copy,
                    )

                bc1_chunk(0)

                xshT = cp.tile([DN, 1, NSH], BF16)
                xof = stp.tile([DN, NSH], F32, tag="xof", bufs=1)
                nc.sync.dma_start(out=xof[:], in_=xoT[:])
                nc.vector.tensor_copy(out=xshT[:, 0, :], in_=xof[:])
                a2f = stp.tile([DE, e2], F32, tag="a1f", bufs=1, name="a2f")
                nc.sync.dma_start(out=a2f[:], in_=a2T[:])
                a2b = stp.tile([DE, e2], BF16, tag="a1b", bufs=1, name="a2b")
                nc.vector.tensor_copy(out=a2b[:], in_=a2f[:])
                attrT2_dram = dr.tile([DE, e2], BF16)
                nc.sync.dma_start(out=attrT2_dram[:], in_=a2b[:])

                sp_block(0, 0)
                w1_chunk(0)
                bc1_chunk(1)
                for c in range(1, 4):
                    w1_chunk(c)

                nc.sync.dma_start(out=cpk[:], in_=cpackA[:])

                # packed small weights (one DMA + one convert on Act);
                # also pre-warm the sigmoid activation table off-critical-path
                nc.sync.dma_start(out=cpkB[:], in_=cpackB[:])
                nc.scalar.activation(out=wbf[:], in_=cpkB[:], func=AF.Copy)
                sgw = wp.tile([1, 1], F32, tag="sgw", bufs=1)
                nc.scalar.activation(
                    out=sgw[:], in_=CF("l2b", 0, 1, 0, 1), func=AF.Sigmoid
                )

                ones = cp.tile([1, P], F32)
                nc.vector.memset(ones[:], 1.0)

                with tc.tile_pool(name="psA", bufs=1, space="PSUM") as psA:
                    # ======== conv1: msg1 = z1 @ W1' + x[src] @ B1'
                    msg_ps = [
                        psA.tile([P, 2 * H], F32, space="PSUM",
                                 tag=f"msg{j}", name=f"msg1_{j}")
                        for j in range((ET1 + 1) // 2)
                    ]

                    def m1(e):
                        return msg_ps[e // 2][:, (e % 2) * H : (e % 2) * H + H]

                    for t in range(16):
                        zt = wp.tile([P, e1], BF16, tag="zt", bufs=6)
                        nc.vector.tensor_tensor(
                            out=zt[:], in0=xsrcT[:, 0, :],
                            in1=bcb[t // 8][:, t % 8, 0:e1], op=ALU.mult,
                        )
                        for e in range(ET1):
                            nc.tensor.matmul(
                                m1(e), lhsT=zt[:, P * e : P * (e + 1)],
                                rhs=w1sb[:, t, :],
                                start=(t == 0 and e % 2 == 0), stop=False,
                                skip_group_check=True,
                            )
                    for e in range(ET1):
                        nc.tensor.matmul(
                            m1(e), lhsT=xsrcT[0:DN, 0, P * e : P * (e + 1)],
                            rhs=WF("b1p", 0, H, 0, DN), start=False, stop=True,
                            skip_group_check=True,
                        )

                    # conv2 background loads on SP: first group (w2 c0..3 +
                    # bc2 tiles 0,2) gated behind bc1; the rest issued after
                    # the h1 gathers so the DMA FIFO has a gap for the
                    # critical handoff transfers
                    sp_block(1, 2)
                    FP8K = []
                    w2sb = cp.tile([P, 2 * DE, H], BF16)
                    w2f8 = (
                        cp.tile([P, 2 * len(FP8K), H], F8) if FP8K else None
                    )
                    w2_src = nn2_w.rearrange("k (h p o) -> p (k h) o", h=2, p=P, o=H)
                    w2st = []

                    def w2_dma(c):
                        st = wp.tile(
                            [P, 4, H], F32, tag=f"w2st{c % 4}", bufs=1,
                            name=f"w2st{c}",
                        )
                        nc.sync.dma_start(
                            out=st[:], in_=w2_src[:, 4 * c : 4 * (c + 1), :]
                        )
                        w2st.append(st)

                    def bc2_dma(kc):
                        nc.sync.dma_start(
                            out=bcb[kc][:, :, 0:e2],
                            in_=attrT2_dram[8 * kc : 8 * (kc + 1), :]
                            .partition_broadcast(P),
                        )

                    for c in range(4):
                        w2_dma(c)
                    bc2_dma(0)
                    bc2_dma(2)

                    # scatter msg1 to own nodes + root + bias, relu -> h1
                    agg_ps = [
                        psA.tile([P, 2 * H], F32, space="PSUM",
                                 tag=f"agg{j}", name=f"agg1_{j}")
                        for j in range(NT // 2)
                    ]

                    def a1(n):
                        return agg_ps[n // 2][:, (n % 2) * H : (n % 2) * H + H]

                    msbs = []
                    for j in range((ET1 + 1) // 2):
                        w = min(2 * H, (ET1 - 2 * j) * H)
                        msb = wp.tile([P, 2 * H], BF16, tag=f"msb{j}", bufs=1)
                        nc.vector.tensor_copy(out=msb[:, 0:w], in_=msg_ps[j][:, 0:w])
                        msbs.append(msb)
                    for e in range(ET1):
                        for n in range(NT):
                            oh = wp.tile([P, P], BF16, tag="oh", bufs=6)
                            nc.vector.tensor_scalar(
                                out=oh[:], in0=CF("io512", P * n, P),
                                scalar1=CF("dstl", e, 1), scalar2=None,
                                op0=ALU.is_equal,
                            )
                            nc.tensor.matmul(
                                a1(n), lhsT=oh[:],
                                rhs=msbs[e // 2][:, (e % 2) * H : (e % 2) * H + H],
                                start=(e == 0 and n % 2 == 0), stop=False,
                                skip_group_check=True,
                            )
                    for n in range(NT):
                        nc.tensor.matmul(
                            a1(n), lhsT=xshT[0:DN, 0, P * n : P * (n + 1)],
                            rhs=WF("r1w", 0, H, 0, DN), start=False, stop=False,
                            skip_group_check=True,
                        )
                        nc.tensor.matmul(
                            a1(n), lhsT=ones[:], rhs=CF("b1", 0, H, 0, 1),
                            start=False, stop=True, skip_group_check=True,
                        )
                    h1sb = bp.tile([P, NT, H], BF16)
                    for j in range(NT // 2):
                        nc.scalar.activation(
                            out=h1sb[:, 2 * j : 2 * j + 2, :],
                            in_=agg_ps[j][:, 0 : 2 * H], func=AF.Relu,
                        )
                    # ordering guard: Pool's in-order SEQ reaches the h1
                    # gathers only after this read of h1sb, which waits on the
                    # relu writes (the SBUF-source gather itself does not
                    # data-depend on h1sb)
                    h1g = wp.tile([1, 1], BF16, tag="h1g", bufs=1)
                    nc.gpsimd.tensor_copy(out=h1g[:], in_=h1sb[0:1, 0, 0:1])



                    if upto == "h1":
                        dh = dbg_out("d_h1", [P, NT * H])
                        tmp = wp.tile([P, NT, H], F32, tag="dbgf")
                        nc.vector.tensor_copy(out=tmp[:], in_=h1sb[:])
                        nc.sync.dma_start(
                            out=dh[:].rearrange("p (t o) -> p t o", o=H),
                            in_=tmp[:],
                        )

                    if ST >= 3:
                        # ======== conv2: msg2 = z2 @ W2' + h1[src] @ B2'
                        h1srcT = bp.tile([P, 2, e2], BF16)
                        nc.gpsimd.dma_gather(
                            out_ap=h1srcT[:], in_ap=h1sb[:],
                            idxs_ap=IF("src2", e2 // 16),
                            num_idxs=e2, num_idxs_reg=e2, elem_size=H,
                            transpose=True, single_packet=False,
                            sbuf_tokens_per_rank=P,
                            sbuf_free_dim_per_rank=H * 2,
                        )
                        h1shT = bp.tile([P, 2, NSH], BF16)
                        nc.gpsimd.dma_gather(
                            out_ap=h1shT[:], in_ap=h1sb[:],
                            idxs_ap=IF("loc", NSH // 16),
                            num_idxs=NSH, num_idxs_reg=NSH, elem_size=H,
                            transpose=True, single_packet=False,
                            sbuf_tokens_per_rank=P,
                            sbuf_free_dim_per_rank=H * 2,
                        )

                        # pre-generate pooling one-hots while DVE is idle
                        ohgs = []
                        for e in range(ET2):
                            pair = []
                            for g in range(2):
                                t_ = wp.tile([P, P], BF16, tag=f"ohg{e}_{g}",
                                             bufs=1)
                                nc.vector.tensor_scalar(
                                    out=t_[:], in0=CF("iog", P * g, P),
                                    scalar1=CF("gl", e, 1), scalar2=None,
                                    op0=ALU.is_equal,
                                )
                                pair.append(t_)
                            ohgs.append(pair)
                        ohbs = []
                        for n in range(NT):
                            pair = []
                            for g in range(2):
                                t_ = wp.tile([P, P], BF16, tag=f"ohb{n}_{g}",
                                             bufs=1)
                                nc.vector.tensor_scalar(
                                    out=t_[:], in0=CF("iog", P * g, P),
                                    scalar1=CF("batchl", n, 1), scalar2=None,
                                    op0=ALU.is_equal,
                                )
                                pair.append(t_)
                            ohbs.append(pair)

                        # blocker: SP waits for the h1srcT gather so the DMA
                        # FIFO has a gap for the critical handoff transfers
                        nc.sync.dma_start(
                            out=sink[0:1, 3:4], in_=h1srcT[0:1, 0, 0:1]
                        )
                        bc2_dma(1)
                        for c in range(4, 8):
                            w2_dma(c)
                        bc2_dma(3)
                        for c in range(8, 16):
                            w2_dma(c)
                        # bf16 conversions on Pool (program-after the gathers,
                        # so the in-order Pool queue does the handoff first)
                        for c in range(16):
                            if not (2 * c in FP8K and 2 * c + 1 in FP8K):
                                nc.gpsimd.tensor_copy(
                                    out=w2sb[:, 4 * c : 4 * (c + 1), :],
                                    in_=w2st[c][:],
                                )
                        # fp8 copies of the DoubleRow chunks (on Act),
                        # pre-scaled by 16 to compensate the 1/16 z scaling
                        # that keeps z inside e4m3 range
                        for i, k in enumerate(FP8K):
                            c, half = k // 2, k % 2
                            nc.scalar.activation(
                                out=w2f8[:, 2 * i : 2 * i + 2, :],
                                in_=w2st[c][:, 2 * half : 2 * half + 2, :],
                                func=AF.Copy, scale=16.0,
                            )

                        msg2_ps = [
                            psA.tile([P, 2 * H], F32, space="PSUM",
                                     tag=f"msg{j}", name=f"msg2_{j}")
                            for j in range((ET2 + 1) // 2)
                        ]

                        def m2(e):
                            return msg2_ps[e // 2][:, (e % 2) * H : (e % 2) * H + H]

                        # attr columns are host-permuted: original dim k lives
                        # at row POS[k] of attrT2 / bc2
                        POS = [k // 2 if k % 2 == 0 else 16 + k // 2
                               for k in range(DE)]
                        for k in range(DE):
                            r = POS[k]
                            if k in FP8K:
                                i = FP8K.index(k)
                                z8 = wp.tile([P, 2, e2], F8, tag="zt8", bufs=3)
                                for ih in range(2):
                                    nc.vector.scalar_tensor_tensor(
                                        out=z8[:, ih, :], in0=h1srcT[:, ih, :],
                                        scalar=0.0625,
                                        in1=bcb[r // 8][:, r % 8, 0:e2],
                                        op0=ALU.mult, op1=ALU.mult,
                                    )
                                for e in range(ET2):
                                    nc.tensor.matmul(
                                        m2(e),
                                        lhsT=z8[:, :, P * e : P * (e + 1)],
                                        rhs=w2f8[:, 2 * i : 2 * i + 2, :],
                                        start=False, stop=False,
                                        perf_mode=DR,
                                        skip_group_check=True,
                                    )
                                continue
                            for ih in range(2):
                                t = 2 * k + ih
                                zt = wp.tile([P, e2], BF16, tag="zt", bufs=6)
                                nc.vector.tensor_tensor(
                                    out=zt[:], in0=h1srcT[:, ih, :],
                                    in1=bcb[r // 8][:, r % 8, 0:e2], op=ALU.mult,
                                )
                                for e in range(ET2):
                                    nc.tensor.matmul(
                                        m2(e), lhsT=zt[:, P * e : P * (e + 1)],
                                        rhs=w2sb[:, t, :],
                                        start=(t == 0 and e % 2 == 0), stop=False,
                                        skip_group_check=True,
                                    )
                        for e in range(ET2):
                            for ih in range(2):
                                nc.tensor.matmul(
                                    m2(e),
                                    lhsT=h1srcT[:, ih, P * e : P * (e + 1)],
                                    rhs=WF("b2p", ih * H, H), start=False,
                                    stop=(ih == 1), skip_group_check=True,
                                )

                    if ST >= 4:
                        # ======== fused pooling
                        msbs2 = []
                        for j in range((ET2 + 1) // 2):
                            w = min(2 * H, (ET2 - 2 * j) * H)
                            msb = wp.tile([P, 2 * H], BF16, tag=f"msb{j}", bufs=1)
                            nc.vector.tensor_copy(
                                out=msb[:, 0:w], in_=msg2_ps[j][:, 0:w]
                            )
                            msbs2.append(msb)

                        # root2 transform on own nodes (node space)
                        rt_ps = [
                            psA.tile([P, 2 * H], F32, space="PSUM",
                                     tag=f"agg{j}", name=f"rt_{j}")
                            for j in range(NT // 2)
                        ]

                        def rt(n):
                            return rt_ps[n // 2][:, (n % 2) * H : (n % 2) * H + H]

                        for n in range(NT):
                            for kh in range(2):
                                nc.tensor.matmul(
                                    rt(n),
                                    lhsT=h1shT[:, kh, P * n : P * (n + 1)],
                                    rhs=WF("r2w", kh * H, H),
                                    start=(kh == 0 and n % 2 == 0),
                                    stop=(kh == 1), skip_group_check=True,
                                )
                        rtsb = bp.tile([P, NT, H], BF16)
                        for j in range(NT // 2):
                            nc.scalar.activation(
                                out=rtsb[:, 2 * j : 2 * j + 2, :],
                                in_=rt_ps[j][:, 0 : 2 * H], func=AF.Copy,
                            )

                        pool_ps = psA.tile([P, 2 * H], F32, space="PSUM",
                                           tag="msg0", name="pool_ps")

                        def pl(g):
                            return pool_ps[:, g * H : g * H + H]

                        for e in range(ET2):
                            for g in range(2):
                                nc.tensor.matmul(
                                    pl(g), lhsT=ohgs[e][g][:],
                                    rhs=msbs2[e // 2][:, (e % 2) * H : (e % 2) * H + H],
                                    start=(e == 0 and g == 0), stop=False,
                                    skip_group_check=True,
                                )
                        for n in range(NT):
                            for g in range(2):
                                nc.tensor.matmul(
                                    pl(g), lhsT=ohbs[n][g][:], rhs=rtsb[:, n, :],
                                    start=False, stop=False,
                                    skip_group_check=True,
                                )
                        for g in range(2):
                            nc.tensor.matmul(
                                pl(g), lhsT=CF("cnt", P * g, P, 0, 1),
                                rhs=CF("b2", 0, H, 0, 1), start=False, stop=True,
                                skip_group_check=True,
                            )

                        nc.scalar.activation(
                            out=sgw[:], in_=CF("l2b", 0, 1, 0, 1), func=AF.Sigmoid
                        )
                        plsb = bp.tile([P, 2, H], F32)
                        nc.scalar.activation(
                            out=plsb[:], in_=pool_ps[:, 0 : 2 * H], func=AF.Copy
                        )
                        pcc_in = dr.tile([N_GRAPHS, H], F32)
                        nc.gpsimd.dma_start(
                            out=pcc_in[:].rearrange("(g p) c -> p g c", p=P),
                            in_=plsb[:],
                        )

                        if upto == "pool":
                            dp = dbg_out("d_pool", [P, 2 * H])
                            tmpp = wp.tile([P, 2 * H], F32, tag="dbgm")
                            nc.vector.tensor_copy(out=tmpp[:], in_=plsb[:])
                            nc.sync.dma_start(out=dp[:], in_=tmpp[:])

            if ST >= 5:
                pcc_out = dr.tile([GPC, H], F32)
                nc.gpsimd.collective_compute(
                    "ReduceScatter", ALU.add, replica_groups=rg,
                    ins=[pcc_in[:].opt()], outs=[pcc_out[:].opt()],
                )

                # ======== readout MLP on this core's 32 graphs
                with tc.tile_pool(name="psB", bufs=1, space="PSUM") as psB:
                    rs_sb = bp.tile([GPC, H], F32)
                    nc.sync.dma_start(out=rs_sb[:], in_=pcc_out[:])
                    # transpose + mean-divide fused: poolT = rs_sb.T @ diag(invc)
                    ivd = cpkB[0:GPC, L["ivd"] - WB0 : L["ivd"] - WB0 + GPC]
                    poolT = bp.tile([P, 2, GPC], BF16)
                    for kh in range(2):
                        tp = psB.tile([P, GPC], F32, space="PSUM", tag="tp")
                        nc.tensor.matmul(
                            tp[:], lhsT=rs_sb[:, P * kh : P * (kh + 1)],
                            rhs=ivd, start=True, stop=True,
                        )
                        nc.scalar.activation(
                            out=poolT[:, kh, :], in_=tp[:], func=AF.Copy
                        )
                    z1_ps = psB.tile([P, GPC], F32, space="PSUM", tag="z1")
                    for kh in range(2):
                        nc.tensor.matmul(
                            z1_ps[:], lhsT=WF("l1w", kh * (H // 2), H // 2),
                            rhs=poolT[:, kh, :],
                            start=(kh == 0), stop=(kh == 1),
                        )
                    z1sb = bp.tile([P, GPC], BF16)
                    nc.scalar.activation(
                        out=z1sb[:], in_=z1_ps[:], func=AF.Relu,
                        bias=CF("l1b", 0, 1, 0, H // 2),
                    )
                    o_ps = psB.tile([1, GPC], F32, space="PSUM", tag="op")
                    nc.tensor.matmul(
                        o_ps[:], lhsT=WF("l2w", 0, 1, 0, H // 2), rhs=z1sb[:],
                        start=True, stop=True,
                    )
                    osb = bp.tile([1, GPC], F32)
                    nc.scalar.activation(
                        out=osb[:], in_=o_ps[:], func=AF.Sigmoid,
                        bias=CF("l2b", 0, 1, 0, 1),
                    )
                    nc.sync.dma_start(
                        out=out[:].rearrange("g one -> one g"), in_=osb[:]
                    )

    nc.compile()
    return nc


def _partition_nodes(src, dst):
    """Greedy balanced node partition: 512 nodes/core minimizing
    max(conv1 load, conv2 load) per core."""
    din = np.bincount(dst, minlength=N_NODES)
    dout = np.bincount(src, minlength=N_NODES)
    order = np.argsort(-(din + dout), kind="stable")
    l1 = np.zeros(NCORES)
    l2 = np.zeros(NCORES)
    slots = np.full(NCORES, NSH)
    owner = np.empty(N_NODES, np.int64)
    for n in order:
        cand = np.nonzero(slots > 0)[0]
        cost = np.maximum(l1[cand] + din[n], l2[cand] + dout[n])
        c = cand[np.argmin(cost)]
        owner[n] = c
        l1[c] += din[n]
        l2[c] += dout[n]
        slots[c] -= 1
    return owner


def _prep_inputs(inputs):
    x = np.asarray(inputs["x"], dtype=np.float32)
    ei = np.asarray(inputs["edge_index"])
    attr = np.asarray(inputs["edge_attr"], dtype=np.float32)
    batch = np.asarray(inputs["batch"]).astype(np.int64)
    src, dst = ei[0].astype(np.int64), ei[1].astype(np.int64)

    owner = _partition_nodes(src, dst)
    nodes = [np.nonzero(owner == c)[0] for c in range(NCORES)]
    g2l = np.zeros(N_NODES, np.int64)
    for c in range(NCORES):
        g2l[nodes[c]] = np.arange(NSH)

    e1s = [np.nonzero(owner[dst] == c)[0] for c in range(NCORES)]
    e2s = [np.nonzero(owner[src] == c)[0] for c in range(NCORES)]
    e1 = max(((max(len(e) for e in e1s) + P - 1) // P) * P, P)
    e2 = max(((max(len(e) for e in e2s) + P - 1) // P) * P, P)
    L = _lay(e1, e2)
    ET1, ET2 = e1 // P, e2 // P

    cnt_glob = np.bincount(batch, minlength=N_GRAPHS).astype(np.float32)
    inv_glob = 1.0 / np.maximum(cnt_glob, 1.0)

    perm = np.concatenate([np.arange(0, DE, 2), np.arange(1, DE, 2)])

    # shared part of cpack (weights + iotas)
    cbase = np.zeros((P, L["cw"]), np.float32)
    cbase[:, L["io512"] : L["io512"] + NSH] = np.arange(NSH, dtype=np.float32)
    cbase[:, L["iog"] : L["iog"] + N_GRAPHS] = np.arange(
        N_GRAPHS, dtype=np.float32
    )
    cbase[0 : H // 2, L["l1b"]] = np.asarray(
        inputs["lin1_b"], dtype=np.float32
    ).ravel()
    cbase[0, L["b1"] : L["b1"] + H] = np.asarray(inputs["bias1"], np.float32).ravel()
    cbase[0, L["b2"] : L["b2"] + H] = np.asarray(inputs["bias2"], np.float32).ravel()
    cbase[0, L["l2b"]] = float(np.asarray(inputs["lin2_b"]).ravel()[0])
    cbase[0:DN, L["b1p"] : L["b1p"] + H] = np.asarray(
        inputs["nn1_b"], np.float32
    ).reshape(DN, H)
    cbase[0:DN, L["r1w"] : L["r1w"] + H] = np.asarray(inputs["root1_w"], np.float32)
    cbase[:, L["b2p"] : L["b2p"] + 2 * H] = (
        np.asarray(inputs["nn2_b"], np.float32)
        .reshape(2, P, H).transpose(1, 0, 2).reshape(P, 2 * H)
    )
    cbase[:, L["r2w"] : L["r2w"] + 2 * H] = (
        np.asarray(inputs["root2_w"], np.float32)
        .reshape(2, P, H).transpose(1, 0, 2).reshape(P, 2 * H)
    )
    cbase[:, L["l1w"] : L["l1w"] + H] = (
        np.asarray(inputs["lin1_w"], np.float32)
        .reshape(2, P, H // 2).transpose(1, 0, 2).reshape(P, H)
    )
    cbase[0 : H // 2, L["l2w"]] = np.asarray(inputs["lin2_w"], np.float32).ravel()
    cbase[:, L["ident"] : L["ident"] + P] = np.eye(P, dtype=np.float32)

    attr_perm = np.ascontiguousarray(attr[:, perm].astype(np.float32))
    common = {
        "nn1_w": np.asarray(inputs["nn1_w"], dtype=np.float32),
        "nn2_w": np.asarray(inputs["nn2_w"], dtype=np.float32),
    }

    in_maps = []
    for c in range(NCORES):
        eids1 = e1s[c]
        n1 = len(eids1)
        src1 = np.zeros(e1, dtype=np.int16)
        src1[:n1] = src[eids1]
        eid1 = np.zeros(e1, dtype=np.int16)
        eid1[:n1] = eids1
        dstl_c = np.full(e1, -1.0, dtype=np.float32)
        dstl_c[:n1] = g2l[dst[eids1]].astype(np.float32)

        eids2 = e2s[c]
        n2 = len(eids2)
        src2 = np.zeros(e2, dtype=np.int16)
        src2[:n2] = g2l[src[eids2]]
        eid2 = np.zeros(e2, dtype=np.int16)
        eid2[:n2] = eids2
        gl_c = np.full(e2, -1.0, dtype=np.float32)
        gl_c[:n2] = batch[dst[eids2]].astype(np.float32)

        cpk = cbase.copy()
        cpk[:, L["dstl"] : L["dstl"] + ET1] = dstl_c.reshape(ET1, P).T
        cpk[:, L["gl"] : L["gl"] + ET2] = gl_c.reshape(ET2, P).T
        cpk[:, L["batchl"] : L["batchl"] + NT] = (
            batch[nodes[c]].astype(np.float32).reshape(NT, P).T
        )
        cpk[0:GPC, L["invc"]] = inv_glob[c * GPC : (c + 1) * GPC]
        cpk[0:GPC, L["ivd"] : L["ivd"] + GPC] = np.diag(
            inv_glob[c * GPC : (c + 1) * GPC]
        )
        cpk[0, L["cnt"] : L["cnt"] + N_GRAPHS] = np.bincount(
            batch[nodes[c]], minlength=N_GRAPHS
        ).astype(np.float32)

        ipk = np.zeros((P, L["iw"]), np.int16)

        def put(name, vals, n):
            ipk[:, L[name] : L[name] + n // 16] = _wrap_idx(vals, n)

        put("src2", src2, e2)
        put("loc", np.arange(NSH, dtype=np.int16), NSH)

        m = dict(common)
        m["cpackA"] = np.ascontiguousarray(cpk[:, : L["wb0"]])
        m["cpackB"] = np.ascontiguousarray(cpk[:, L["wb0"] :])
        m["ipack"] = ipk
        m["a1T"] = np.ascontiguousarray(attr_perm[eid1.astype(np.int64), :].T)
        m["a2T"] = np.ascontiguousarray(attr_perm[eid2.astype(np.int64), :].T)
        xs = x[src1.astype(np.int64), :].T.astype(np.float32)
        m["x1T"] = np.ascontiguousarray(np.vstack([xs, xs]))
        m["xoT"] = np.ascontiguousarray(x[nodes[c], :].T.astype(np.float32))
        in_maps.append(m)
    return e1, e2, in_maps


def kernel(**inputs) -> np.ndarray:
    e1, e2, in_maps = _prep_inputs(inputs)
    if (e1, e2) not in _cache:
        _cache[(e1, e2)] = _build(e1, e2)
    nc = _cache[(e1, e2)]
    res = bass_utils.run_bass_kernel_spmd(nc, in_maps, core_ids=list(range(NCORES)))
    return np.concatenate(
        [np.asarray(res.results[c]["out"], dtype=np.float32) for c in range(NCORES)],
        axis=0,
    )


def run_debug(upto, **inputs):
    e1, e2, in_maps = _prep_inputs(inputs)
    nc = _build(e1, e2, upto=upto)
    res = bass_utils.run_bass_kernel_spmd(nc, in_maps, core_ids=list(range(NCORES)))
    return e1, e2, res
